# revision 19
# baseline (speedup 1.0000x reference)
"""Trainium2 Bass kernel for KernelPointAggregation (hyperbolic GNN message passing).

v4 strategy: SBUF XF table + one-hot-matmul neighbor aggregation (no gather).
-----------------------------------------------------------------------------
Every per-edge quantity factorizes per source node j = nei[n, m]: the kernel
is a per-node table XF[j, :] followed by a masked sum over neighbors and a
Lorentz normalize (the normalize cancels any per-node weight scale, so the
{1e-4, 1.0001} mask weights reduce to {drop, keep} -> a 0/1-count matrix A).

The neighbor aggregation agg = A^T @ XF runs on the (otherwise idle) tensor
engine, accumulated rank-by-rank INSIDE phase 1: as soon as chunk c writes
table ranks 12c..12c+11, the PE contracts those ranks against the streamed
A rows, so phase 2 costs no extra wall time. A (12288 x 1536 bf16 per core,
~38 MB) streams from HBM overlapped with phase-1 compute.

Per core:
  Phase 1 (replicated): 8 chunks x 12 tiles x 128 nodes:
    y[j,(k,o)] = [x_j,1] @ wtb,  nis[j,k] = [x_j,1] @ kmtb,
    kernel-softmax collapses to u^-1 = z - sqrt(z^2-1) (denominator cancels),
    k-aggregation + Lorentz normalize -> XF row -> SBUF table bf16
    (partition = j%128, rank j//128; cols 64..127 of each rank block = 0).
  Interleaved per chunk: for each new rank r, 3 matmuls accumulate
    psum[64, 512-slice] += table[:, r-block]^T @ A[r-block rows].
  Tail: cast psum -> bf16, PE-transpose [64, n] -> [n, 64], Lorentz midpoint
    normalize, DMA out.
"""

import sys

sys.path.insert(0, "/opt/trn_rl_repo")

import math
import os

import numpy as np

import concourse.bacc as bacc
import concourse.mybir as mybir
import concourse.tile as tile
from concourse.bass_utils import run_bass_kernel_spmd

N, NEI, K, D = 12000, 32, 8, 64
NCORES = 8
SLICE = N // NCORES          # 1500 nodes per core (phase 2 ownership)
NT = 12                      # node tiles per core slice (128 each)
NPAD = NT * 128              # 1536 padded rows per core slice
NPF = 12288                  # padded full node count (96 ranks of 128)
NRANKS = 96
NCHUNK = 8                   # phase-1 chunks
TCH = 12                     # tiles per chunk
CH = TCH * 128               # 1536 nodes per chunk
KD = K * D                   # 512
S = TCH * K                  # 96 (t,k) pairs per chunk
HS = S // 2                  # 48
HKD = TCH * KD // 2          # 3072
SG = 68                      # DVE (t,k) groups of S=96; GpSimd takes 28
SPL = SG * D                 # 5120
F32 = mybir.dt.float32
BF16 = mybir.dt.bfloat16
AX = mybir.AxisListType
OP = mybir.AluOpType
AF = mybir.ActivationFunctionType


def _build_program():
    nc = bacc.Bacc("TRN2", target_bir_lowering=False, debug=False,
                   num_devices=NCORES)

    xaugbf_in = nc.dram_tensor("xaugbf", [D + 1, NPF], BF16,
                               kind="ExternalInput")
    wtb_in = nc.dram_tensor("wtbbf", [D + 1, KD], BF16, kind="ExternalInput")
    kmtb_in = nc.dram_tensor("kmtb", [D + 1, K], BF16, kind="ExternalInput")
    es_in = nc.dram_tensor("es96", [128, S], F32, kind="ExternalInput")
    amat_in = nc.dram_tensor("amat", [128, NRANKS * NPAD], BF16,
                             kind="ExternalInput")
    id_in = nc.dram_tensor("ident", [D, D], BF16, kind="ExternalInput")
    out_dram = nc.dram_tensor("out", [NPAD, D], F32, kind="ExternalOutput")

    with tile.TileContext(nc) as tc:
        with (
            tc.tile_pool(name="const", bufs=1) as cpool,
            tc.tile_pool(name="xa", bufs=3) as xpool,
            tc.tile_pool(name="amat", bufs=4) as apool,
            tc.tile_pool(name="yb", bufs=2) as bpool,
            tc.tile_pool(name="sq", bufs=2) as sqpool,
            tc.tile_pool(name="tm", bufs=2) as tmpool,
            tc.tile_pool(name="scr", bufs=3) as spool,
            tc.tile_pool(name="nrm", bufs=2) as nrmpool,
            tc.tile_pool(name="psum", bufs=1, space="PSUM") as ppool,
            tc.tile_pool(name="psumn", bufs=1, space="PSUM") as npool,
            tc.tile_pool(name="psuma", bufs=1, space="PSUM") as acpool,
            tc.tile_pool(name="psumt", bufs=2, space="PSUM") as tpool,
        ):
            # ---- constants ----
            wtb = cpool.tile([D + 1, KD], BF16)
            nc.sync.dma_start(wtb[:], wtb_in[:])
            kmtb = cpool.tile([D + 1, K], BF16)
            nc.sync.dma_start(kmtb[:], kmtb_in[:])
            es96 = cpool.tile([128, S], F32)
            nc.sync.dma_start(es96[:], es_in[:])
            ident = cpool.tile([D, D], BF16)
            nc.sync.dma_start(ident[:], id_in[:])

            # SBUF XF table: partition = j%128, rank j//128 at 256B stride;
            # cols 0..63 of each 128-wide rank block = XF row, 64..127 = 0.
            biasm1 = cpool.tile([128, 1], F32)
            nc.vector.memset(biasm1[:], -1.0)
            table = cpool.tile([128, NRANKS * 128], BF16)
            nc.gpsimd.memset(table[:], 0.0)

            # phase-2 accumulators: agg.T[64, 1536] in 3 psum banks
            accps = [acpool.tile([D, 512], F32, tag=f"acc{s}",
                                 name=f"accps{s}")
                     for s in range(3)]

            # ================= Phase 1 + interleaved A-matmul ============
            for c in range(NCHUNK):
                xaugbf = xpool.tile([D + 1, CH], BF16, tag="xaugbf")
                nc.sync.dma_start(xaugbf[:],
                                  xaugbf_in[:, c * CH:(c + 1) * CH])
                ybuf = bpool.tile([128, TCH * KD], BF16, tag="ybuf")
                sqb = sqpool.tile([128, TCH * KD], BF16, tag="sqb")
                nis = spool.tile([128, S], F32, tag="nis")
                ssqn = spool.tile([128, S], F32, tag="ssqn")
                sig = spool.tile([128, S], F32, tag="sig")
                nipb = npool.tile([128, S], F32, tag="nip")

                for tt in range(0, TCH, 2):
                    yp2 = ppool.tile([128, 2 * KD], F32, tag="yp")
                    for u in range(2):
                        t = tt + u
                        nc.tensor.matmul(yp2[:, u * KD:(u + 1) * KD],
                                         xaugbf[:, t * 128:(t + 1) * 128],
                                         wtb[:], start=True, stop=True)
                        nc.tensor.matmul(nipb[:, t * K:(t + 1) * K],
                                         xaugbf[:, t * 128:(t + 1) * 128],
                                         kmtb[:], start=True, stop=True)
                    nc.scalar.copy(ybuf[:, tt * KD:(tt + 2) * KD], yp2[:])
                nc.vector.tensor_copy(nis[:], nipb[:])

                # squares + segmented ||nar||^2, split DVE / GpSimd halves
                nc.vector.tensor_tensor(sqb[:, :SPL], ybuf[:, :SPL],
                                        ybuf[:, :SPL], op=OP.mult)
                nc.gpsimd.tensor_tensor(sqb[:, SPL:], ybuf[:, SPL:],
                                        ybuf[:, SPL:], op=OP.mult)
                sqbk = sqb[:].rearrange("p (s o) -> p s o", s=S)
                ssqa = spool.tile([128, S], F32, tag="ssqa")
                nc.vector.tensor_reduce(ssqa[:], sqbk, axis=AX.X, op=OP.add)
                nc.vector.tensor_tensor(ssqn[:], ssqa[:], sqbk[:, :, 0],
                                        op=OP.subtract)

                ybk = ybuf[:].rearrange("p (s o) -> p s o", s=S)
                nc.scalar.activation(sig[:], ybk[:, :, 0], AF.Sigmoid)
                # ---- batched nonlinear over [128, 96] ----
                # 1/u = 1/(z+sqrt(z^2-1)) = z - sqrt(z^2-1)
                z = spool.tile([128, S], F32, tag="pb1")
                nc.vector.tensor_scalar_max(z[:], nis[:], 1.0 + 1e-7)
                zsq = spool.tile([128, S], F32, tag="pb2")
                nc.vector.tensor_tensor(zsq[:], z[:], z[:], op=OP.mult)
                sq1 = spool.tile([128, S], F32, tag="pb3")
                nc.scalar.activation(sq1[:], zsq[:], AF.Sqrt,
                                     bias=biasm1[:])
                uinv = spool.tile([128, S], F32, tag="pb5")
                nc.vector.tensor_tensor(uinv[:], z[:], sq1[:],
                                        op=OP.subtract)
                tt2 = spool.tile([128, S], F32, tag="pb9")
                nc.vector.tensor_tensor(tt2[:], sig[:], es96[:], op=OP.mult)
                nc.vector.tensor_scalar_add(tt2[:], tt2[:], 1.0001)
                num = spool.tile([128, S], F32, tag="pb10")
                nc.vector.tensor_tensor(num[:], tt2[:], tt2[:], op=OP.mult)
                nc.vector.tensor_scalar_sub(num[:], num[:], 1.0)
                dnn = spool.tile([128, S], F32, tag="pb11")
                nc.vector.tensor_scalar_max(dnn[:], ssqn[:], 1e-8)
                dni = spool.tile([128, S], F32, tag="pb12")
                nc.vector.reciprocal(dni[:], dnn[:])
                sc = spool.tile([128, S], F32, tag="pb13")
                nc.vector.tensor_tensor(sc[:], num[:], dni[:], op=OP.mult)
                rt = spool.tile([128, S], F32, tag="pb14")
                nc.scalar.activation(rt[:], sc[:], AF.Sqrt)
                csp = spool.tile([128, S], BF16, tag="pb15")
                nc.vector.tensor_tensor(csp[:], uinv[:], rt[:], op=OP.mult)
                wt = spool.tile([128, S], F32, tag="pb16")
                nc.vector.tensor_tensor(wt[:], uinv[:], tt2[:], op=OP.mult)
                aggt = spool.tile([128, TCH], F32, tag="pb17")
                nc.vector.tensor_reduce(
                    aggt[:], wt[:].rearrange("p (t k) -> p t k", k=K),
                    axis=AX.X, op=OP.add)

                # ---- k-aggregation: weight, then pairwise-add over k ----
                tmpb = tmpool.tile([128, TCH * KD], BF16, tag="tmpb")
                nc.vector.tensor_tensor(
                    tmpb[:, :SPL].rearrange("p (s o) -> p s o", s=SG),
                    ybk[:, :SG, :],
                    csp[:, :SG].to_broadcast([128, SG, D]), op=OP.mult)
                nc.gpsimd.tensor_tensor(
                    tmpb[:, SPL:].rearrange("p (s o) -> p s o", s=S - SG),
                    ybk[:, SG:, :],
                    csp[:, SG:].to_broadcast([128, S - SG, D]), op=OP.mult)
                tk = tmpb[:].rearrange("p (t k o) -> p t k o", t=TCH, k=K)
                a1 = sqpool.tile([128, TCH * 4 * D], BF16, tag="a1")
                a1k = a1[:].rearrange("p (t k o) -> p t k o", t=TCH, k=4)
                nc.vector.tensor_tensor(a1k, tk[:, :, 0:4, :],
                                        tk[:, :, 4:8, :], op=OP.add)
                a2 = sqpool.tile([128, TCH * 2 * D], BF16, tag="a2")
                a2k = a2[:].rearrange("p (t k o) -> p t k o", t=TCH, k=2)
                nc.vector.tensor_tensor(a2k, a1k[:, :, 0:2, :],
                                        a1k[:, :, 2:4, :], op=OP.add)
                agg = nrmpool.tile([128, TCH * D], F32, tag="agg")
                agk = agg[:].rearrange("p (t o) -> p t o", o=D)
                nc.vector.tensor_tensor(agk, a2k[:, :, 0, :],
                                        a2k[:, :, 1, :], op=OP.add)
                nc.vector.tensor_copy(agk[:, :, 0], aggt[:])

                # ---- Lorentz normalize, write bf16 into SBUF table ----
                sqa = nrmpool.tile([128, TCH * D], F32, tag="sqa")
                nc.scalar.square(sqa[:], agg[:])
                sqak = sqa[:].rearrange("p (t o) -> p t o", o=D)
                sp2 = spool.tile([128, TCH], F32, tag="sp2")
                nc.vector.tensor_reduce(sp2[:], sqak, axis=AX.X, op=OP.add)
                inner = spool.tile([128, TCH], F32, tag="inner")
                nc.vector.scalar_tensor_tensor(
                    inner[:], sqak[:, :, 0], -2.0, sp2[:],
                    op0=OP.mult, op1=OP.add)
                nc.vector.scalar_tensor_tensor(
                    inner[:], inner[:], -1.0, inner[:],
                    op0=OP.mult, op1=OP.max)
                nc.vector.tensor_scalar_max(inner[:], inner[:], 1e-8)
                rden = spool.tile([128, TCH], F32, tag="rden")
                nc.scalar.sqrt(rden[:], inner[:])
                rinv = spool.tile([128, TCH], F32, tag="rinv")
                nc.vector.reciprocal(rinv[:], rden[:])
                tview = table[:, c * TCH * 128:(c + 1) * TCH * 128].rearrange(
                    "p (t w) -> p t w", w=128)[:, :, 0:D]
                nc.vector.tensor_tensor(
                    tview, agk, rinv[:].to_broadcast([128, TCH, D]),
                    op=OP.mult)

                # ---- interleaved phase 2: accumulate agg.T += XF_r^T A_r --
                for r12 in range(TCH):
                    r = c * TCH + r12
                    ar = apool.tile([128, NPAD], BF16, tag="ar")
                    nc.sync.dma_start(ar[:],
                                      amat_in[:, r * NPAD:(r + 1) * NPAD])
                    for s in range(3):
                        nc.tensor.matmul(
                            accps[s][:], table[:, r * 128:r * 128 + D],
                            ar[:, s * 512:(s + 1) * 512],
                            start=(r == 0), stop=(r == NRANKS - 1))

            # ================= Phase 2 tail: transpose + normalize ========
            accb = cpool.tile([D, NPAD], BF16)
            for s in range(3):
                nc.vector.tensor_copy(accb[:, s * 512:(s + 1) * 512],
                                      accps[s][:])
            acc = cpool.tile([128, NT * D], F32)
            for t in range(NT):
                tp = tpool.tile([128, D], BF16, tag="p2_tp")
                nc.tensor.transpose(tp[:], accb[:, t * 128:(t + 1) * 128],
                                    ident[:])
                nc.scalar.copy(acc[:, t * D:(t + 1) * D], tp[:])

            sqa2 = spool.tile([128, NT * D], F32, tag="q_sqa")
            nc.scalar.square(sqa2[:], acc[:])
            sqa2k = sqa2[:].rearrange("p (t o) -> p t o", o=D)
            sp22 = spool.tile([128, NT], F32, tag="q_sp2")
            nc.vector.tensor_reduce(sp22[:], sqa2k[:, :, 1:], axis=AX.X,
                                    op=OP.add)
            inner2 = spool.tile([128, NT], F32, tag="q_inner")
            nc.vector.tensor_tensor(inner2[:], sp22[:], sqa2k[:, :, 0],
                                    op=OP.subtract)
            nc.vector.scalar_tensor_tensor(
                inner2[:], inner2[:], -1.0, inner2[:], op0=OP.mult, op1=OP.max)
            nc.vector.tensor_scalar_max(inner2[:], inner2[:], 1e-8)
            rden2 = spool.tile([128, NT], F32, tag="q_rden")
            nc.scalar.sqrt(rden2[:], inner2[:])
            rinv2 = spool.tile([128, NT], F32, tag="q_rinv")
            nc.vector.reciprocal(rinv2[:], rden2[:])
            outb = spool.tile([128, NT * D], F32, tag="q_out")
            nc.vector.tensor_tensor(
                outb[:].rearrange("p (t o) -> p t o", o=D),
                acc[:].rearrange("p (t o) -> p t o", o=D),
                rinv2[:].to_broadcast([128, NT, D]), op=OP.mult)

            nc.sync.dma_start(
                out_dram[:].rearrange("(t p) d -> p t d", p=128),
                outb[:].rearrange("p (t d) -> p t d", t=NT))

    nc.compile()
    return nc


_NC_CACHE = None


def _get_nc():
    global _NC_CACHE
    if _NC_CACHE is None:
        _NC_CACHE = _build_program()
    return _NC_CACHE


def _host_inputs(x, nei, nei_mask, kernel_points, W, b, scales):
    x = np.asarray(x, np.float32)
    nei = np.asarray(nei, np.int64)
    nei_mask = np.asarray(nei_mask)
    kernel_points = np.asarray(kernel_points, np.float32)
    W = np.asarray(W, np.float32)
    b = np.asarray(b, np.float32)
    scales = np.asarray(scales, np.float32)

    # kernels = expmap0(kernel_points), then fold the Lorentz metric
    sp = kernel_points[:, 1:]
    nrm = np.sqrt(np.clip(np.sum(sp * sp, -1, keepdims=True), 1e-8, None)
                  ).astype(np.float32)
    kernels = np.concatenate(
        [np.cosh(nrm), np.sinh(nrm) / nrm * sp], -1).astype(np.float32)
    metric = np.ones((D,), np.float32)
    metric[1:] = -1.0
    km = kernels * metric                      # [K, D]

    xp = np.concatenate([x, x[:NPF - N]], 0)   # [12288, 64]
    xaugT = np.concatenate(
        [xp.T, np.ones((1, NPF), np.float32)], 0).astype(np.float32)

    wtb = np.concatenate(
        [np.transpose(W, (2, 0, 1)).reshape(D, KD), b.reshape(1, KD)],
        0).astype(np.float32)                  # [65, 512]
    kmtb = np.concatenate([km.T, np.zeros((1, K), np.float32)], 0)  # [65, 8]
    es96 = np.tile(np.exp(scales).astype(np.float32), TCH)[None, :].repeat(
        128, 0).copy()                         # [128, 96]
    ident = np.eye(D, dtype=np.float32)

    # Mask folded into the 0/1-count matrix A[j, n_local]; all-masked nodes
    # keep every edge (equal weights == the reference's uniform-1e-4
    # midpoint; the final normalize cancels the scale).
    mask = nei_mask.astype(bool)
    allm = ~mask.any(1)
    eff = mask | allm[:, None]                 # [N, NEI]

    import ml_dtypes
    shared = {"xaugbf": np.ascontiguousarray(xaugT.astype(ml_dtypes.bfloat16)),
              "wtbbf": wtb.astype(ml_dtypes.bfloat16),
              "kmtb": kmtb.astype(ml_dtypes.bfloat16), "es96": es96,
              "ident": ident.astype(ml_dtypes.bfloat16)}
    per_core = []
    for r in range(NCORES):
        nj = nei[r * SLICE:(r + 1) * SLICE]          # [1500, 32]
        ef = eff[r * SLICE:(r + 1) * SLICE]
        nloc = np.repeat(np.arange(SLICE), NEI)
        jsrc = nj.reshape(-1)
        keep = ef.reshape(-1)
        A = np.zeros((NPF, NPAD), np.float32)
        np.add.at(A, (jsrc[keep], nloc[keep]), 1.0)
        amat = np.ascontiguousarray(
            A.reshape(NRANKS, 128, NPAD).transpose(1, 0, 2).reshape(
                128, NRANKS * NPAD).astype(ml_dtypes.bfloat16))
        per_core.append({**shared, "amat": amat})
    return per_core


def kernel(x, nei, nei_mask, kernel_points, W, b, scales, _trace=False):
    nc = _get_nc()
    per_core = _host_inputs(x, nei, nei_mask, kernel_points, W, b, scales)
    res = run_bass_kernel_spmd(nc, per_core, core_ids=list(range(NCORES)),
                               trace=_trace)
    out = np.concatenate(
        [res.results[r]["out"][:SLICE] for r in range(NCORES)], 0)
    if _trace:
        kernel.last_result = res
    return out.astype(np.float32)


# revision 20
# speedup vs baseline: 1.0054x; 1.0054x over previous
"""Trainium2 Bass kernel for KernelPointAggregation (hyperbolic GNN message passing).

v4 strategy: SBUF XF table + one-hot-matmul neighbor aggregation (no gather).
-----------------------------------------------------------------------------
Every per-edge quantity factorizes per source node j = nei[n, m]: the kernel
is a per-node table XF[j, :] followed by a masked sum over neighbors and a
Lorentz normalize (the normalize cancels any per-node weight scale, so the
{1e-4, 1.0001} mask weights reduce to {drop, keep} -> a 0/1-count matrix A).

The neighbor aggregation agg = A^T @ XF runs on the (otherwise idle) tensor
engine, accumulated rank-by-rank INSIDE phase 1: as soon as chunk c writes
table ranks 12c..12c+11, the PE contracts those ranks against the streamed
A rows, so phase 2 costs no extra wall time. A (12288 x 1536 bf16 per core,
~38 MB) streams from HBM overlapped with phase-1 compute.

Per core:
  Phase 1 (replicated): 8 chunks x 12 tiles x 128 nodes:
    y[j,(k,o)] = [x_j,1] @ wtb,  nis[j,k] = [x_j,1] @ kmtb,
    kernel-softmax collapses to u^-1 = z - sqrt(z^2-1) (denominator cancels),
    k-aggregation + Lorentz normalize -> XF row -> SBUF table bf16
    (partition = j%128, rank j//128; cols 64..127 of each rank block = 0).
  Interleaved per chunk: for each new rank r, 3 matmuls accumulate
    psum[64, 512-slice] += table[:, r-block]^T @ A[r-block rows].
  Tail: cast psum -> bf16, PE-transpose [64, n] -> [n, 64], Lorentz midpoint
    normalize, DMA out.
"""

import sys

sys.path.insert(0, "/opt/trn_rl_repo")

import math
import os

import numpy as np

import concourse.bacc as bacc
import concourse.mybir as mybir
import concourse.tile as tile
from concourse.bass_utils import run_bass_kernel_spmd

N, NEI, K, D = 12000, 32, 8, 64
NCORES = 8
SLICE = N // NCORES          # 1500 nodes per core (phase 2 ownership)
NT = 12                      # node tiles per core slice (128 each)
NPAD = NT * 128              # 1536 padded rows per core slice
NPF = 12288                  # padded full node count (96 ranks of 128)
NRANKS = 96
NCHUNK = 8                   # phase-1 chunks
TCH = 12                     # tiles per chunk
CH = TCH * 128               # 1536 nodes per chunk
KD = K * D                   # 512
S = TCH * K                  # 96 (t,k) pairs per chunk
HS = S // 2                  # 48
HKD = TCH * KD // 2          # 3072
SG = 80                      # DVE (t,k) groups of S=96; GpSimd takes 16
SPL = SG * D                 # 5120
F32 = mybir.dt.float32
BF16 = mybir.dt.bfloat16
AX = mybir.AxisListType
OP = mybir.AluOpType
AF = mybir.ActivationFunctionType


def _build_program():
    nc = bacc.Bacc("TRN2", target_bir_lowering=False, debug=False,
                   num_devices=NCORES)

    xaugbf_in = nc.dram_tensor("xaugbf", [D + 1, NPF], BF16,
                               kind="ExternalInput")
    wtb_in = nc.dram_tensor("wtbbf", [D + 1, KD], BF16, kind="ExternalInput")
    kmtb_in = nc.dram_tensor("kmtb", [D + 1, K], BF16, kind="ExternalInput")
    es_in = nc.dram_tensor("es96", [128, S], F32, kind="ExternalInput")
    amat_in = nc.dram_tensor("amat", [128, NRANKS * NPAD], BF16,
                             kind="ExternalInput")
    id_in = nc.dram_tensor("ident", [D, D], BF16, kind="ExternalInput")
    out_dram = nc.dram_tensor("out", [NPAD, D], F32, kind="ExternalOutput")

    with tile.TileContext(nc) as tc:
        with (
            tc.tile_pool(name="const", bufs=1) as cpool,
            tc.tile_pool(name="xa", bufs=3) as xpool,
            tc.tile_pool(name="amat", bufs=4) as apool,
            tc.tile_pool(name="yb", bufs=2) as bpool,
            tc.tile_pool(name="sq", bufs=2) as sqpool,
            tc.tile_pool(name="tm", bufs=2) as tmpool,
            tc.tile_pool(name="scr", bufs=3) as spool,
            tc.tile_pool(name="nrm", bufs=2) as nrmpool,
            tc.tile_pool(name="psum", bufs=1, space="PSUM") as ppool,
            tc.tile_pool(name="psumn", bufs=1, space="PSUM") as npool,
            tc.tile_pool(name="psuma", bufs=1, space="PSUM") as acpool,
            tc.tile_pool(name="psumt", bufs=2, space="PSUM") as tpool,
        ):
            # ---- constants ----
            wtb = cpool.tile([D + 1, KD], BF16)
            nc.sync.dma_start(wtb[:], wtb_in[:])
            kmtb = cpool.tile([D + 1, K], BF16)
            nc.sync.dma_start(kmtb[:], kmtb_in[:])
            es96 = cpool.tile([128, S], F32)
            nc.sync.dma_start(es96[:], es_in[:])
            ident = cpool.tile([D, D], BF16)
            nc.sync.dma_start(ident[:], id_in[:])

            # SBUF XF table: partition = j%128, rank j//128 at 256B stride;
            # cols 0..63 of each 128-wide rank block = XF row, 64..127 = 0.
            table = cpool.tile([128, NRANKS * 128], BF16)
            nc.vector.memset(table[:], 0.0)

            # phase-2 accumulators: agg.T[64, 1536] in 3 psum banks
            accps = [acpool.tile([D, 512], F32, tag=f"acc{s}",
                                 name=f"accps{s}")
                     for s in range(3)]

            # ================= Phase 1 + interleaved A-matmul ============
            for c in range(NCHUNK):
                xaugbf = xpool.tile([D + 1, CH], BF16, tag="xaugbf")
                nc.sync.dma_start(xaugbf[:],
                                  xaugbf_in[:, c * CH:(c + 1) * CH])
                ybuf = bpool.tile([128, TCH * KD], BF16, tag="ybuf")
                sqb = sqpool.tile([128, TCH * KD], BF16, tag="sqb")
                nis = spool.tile([128, S], F32, tag="nis")
                ssqn = spool.tile([128, S], F32, tag="ssqn")
                sig = spool.tile([128, S], F32, tag="sig")
                nipb = npool.tile([128, S], F32, tag="nip")

                for tt in range(0, TCH, 2):
                    yp2 = ppool.tile([128, 2 * KD], F32, tag="yp")
                    for u in range(2):
                        t = tt + u
                        nc.tensor.matmul(yp2[:, u * KD:(u + 1) * KD],
                                         xaugbf[:, t * 128:(t + 1) * 128],
                                         wtb[:], start=True, stop=True)
                        nc.tensor.matmul(nipb[:, t * K:(t + 1) * K],
                                         xaugbf[:, t * 128:(t + 1) * 128],
                                         kmtb[:], start=True, stop=True)
                    nc.scalar.copy(ybuf[:, tt * KD:(tt + 2) * KD], yp2[:])
                nc.vector.tensor_copy(nis[:], nipb[:])

                # squares + segmented ||nar||^2, split DVE / GpSimd halves
                nc.vector.tensor_tensor(sqb[:, :SPL], ybuf[:, :SPL],
                                        ybuf[:, :SPL], op=OP.mult)
                nc.gpsimd.tensor_tensor(sqb[:, SPL:], ybuf[:, SPL:],
                                        ybuf[:, SPL:], op=OP.mult)
                sqbk = sqb[:].rearrange("p (s o) -> p s o", s=S)
                ssqa = spool.tile([128, S], F32, tag="ssqa")
                nc.vector.tensor_reduce(ssqa[:], sqbk, axis=AX.X, op=OP.add)
                nc.vector.tensor_tensor(ssqn[:], ssqa[:], sqbk[:, :, 0],
                                        op=OP.subtract)

                ybk = ybuf[:].rearrange("p (s o) -> p s o", s=S)
                nc.scalar.activation(sig[:], ybk[:, :, 0], AF.Sigmoid)
                # ---- batched nonlinear over [128, 96] ----
                # 1/u = 1/(z+sqrt(z^2-1)) = z - sqrt(z^2-1)
                z = spool.tile([128, S], F32, tag="pb1")
                nc.vector.tensor_scalar_max(z[:], nis[:], 1.0 + 1e-7)
                zsq = spool.tile([128, S], F32, tag="pb2")
                nc.vector.tensor_tensor(zsq[:], z[:], z[:], op=OP.mult)
                nc.vector.tensor_scalar_sub(zsq[:], zsq[:], 1.0)
                sq1 = spool.tile([128, S], F32, tag="pb3")
                nc.scalar.activation(sq1[:], zsq[:], AF.Sqrt)
                uinv = spool.tile([128, S], F32, tag="pb5")
                nc.vector.tensor_tensor(uinv[:], z[:], sq1[:],
                                        op=OP.subtract)
                tt2 = spool.tile([128, S], F32, tag="pb9")
                nc.vector.tensor_tensor(tt2[:], sig[:], es96[:], op=OP.mult)
                nc.vector.tensor_scalar_add(tt2[:], tt2[:], 1.0001)
                num = spool.tile([128, S], F32, tag="pb10")
                nc.vector.tensor_tensor(num[:], tt2[:], tt2[:], op=OP.mult)
                nc.vector.tensor_scalar_sub(num[:], num[:], 1.0)
                dnn = spool.tile([128, S], F32, tag="pb11")
                nc.vector.tensor_scalar_max(dnn[:], ssqn[:], 1e-8)
                dni = spool.tile([128, S], F32, tag="pb12")
                nc.vector.reciprocal(dni[:], dnn[:])
                sc = spool.tile([128, S], F32, tag="pb13")
                nc.vector.tensor_tensor(sc[:], num[:], dni[:], op=OP.mult)
                rt = spool.tile([128, S], F32, tag="pb14")
                nc.scalar.activation(rt[:], sc[:], AF.Sqrt)
                csp = spool.tile([128, S], BF16, tag="pb15")
                nc.vector.tensor_tensor(csp[:], uinv[:], rt[:], op=OP.mult)
                wt = spool.tile([128, S], F32, tag="pb16")
                nc.vector.tensor_tensor(wt[:], uinv[:], tt2[:], op=OP.mult)
                aggt = spool.tile([128, TCH], F32, tag="pb17")
                nc.vector.tensor_reduce(
                    aggt[:], wt[:].rearrange("p (t k) -> p t k", k=K),
                    axis=AX.X, op=OP.add)

                # ---- k-aggregation: weight, then pairwise-add over k ----
                tmpb = tmpool.tile([128, TCH * KD], BF16, tag="tmpb")
                nc.vector.tensor_tensor(
                    tmpb[:, :SPL].rearrange("p (s o) -> p s o", s=SG),
                    ybk[:, :SG, :],
                    csp[:, :SG].to_broadcast([128, SG, D]), op=OP.mult)
                nc.gpsimd.tensor_tensor(
                    tmpb[:, SPL:].rearrange("p (s o) -> p s o", s=S - SG),
                    ybk[:, SG:, :],
                    csp[:, SG:].to_broadcast([128, S - SG, D]), op=OP.mult)
                tk = tmpb[:].rearrange("p (t k o) -> p t k o", t=TCH, k=K)
                a1 = sqpool.tile([128, TCH * 4 * D], BF16, tag="a1")
                a1k = a1[:].rearrange("p (t k o) -> p t k o", t=TCH, k=4)
                nc.vector.tensor_tensor(a1k, tk[:, :, 0:4, :],
                                        tk[:, :, 4:8, :], op=OP.add)
                a2 = sqpool.tile([128, TCH * 2 * D], BF16, tag="a2")
                a2k = a2[:].rearrange("p (t k o) -> p t k o", t=TCH, k=2)
                nc.vector.tensor_tensor(a2k, a1k[:, :, 0:2, :],
                                        a1k[:, :, 2:4, :], op=OP.add)
                agg = nrmpool.tile([128, TCH * D], F32, tag="agg")
                agk = agg[:].rearrange("p (t o) -> p t o", o=D)
                nc.vector.tensor_tensor(agk, a2k[:, :, 0, :],
                                        a2k[:, :, 1, :], op=OP.add)
                nc.vector.tensor_copy(agk[:, :, 0], aggt[:])

                # ---- Lorentz normalize, write bf16 into SBUF table ----
                sqa = nrmpool.tile([128, TCH * D], F32, tag="sqa")
                nc.scalar.square(sqa[:], agg[:])
                sqak = sqa[:].rearrange("p (t o) -> p t o", o=D)
                sp2 = spool.tile([128, TCH], F32, tag="sp2")
                nc.vector.tensor_reduce(sp2[:], sqak, axis=AX.X, op=OP.add)
                inner = spool.tile([128, TCH], F32, tag="inner")
                nc.vector.scalar_tensor_tensor(
                    inner[:], sqak[:, :, 0], -2.0, sp2[:],
                    op0=OP.mult, op1=OP.add)
                nc.vector.scalar_tensor_tensor(
                    inner[:], inner[:], -1.0, inner[:],
                    op0=OP.mult, op1=OP.max)
                nc.vector.tensor_scalar_max(inner[:], inner[:], 1e-8)
                rden = spool.tile([128, TCH], F32, tag="rden")
                nc.scalar.sqrt(rden[:], inner[:])
                rinv = spool.tile([128, TCH], F32, tag="rinv")
                nc.vector.reciprocal(rinv[:], rden[:])
                tview = table[:, c * TCH * 128:(c + 1) * TCH * 128].rearrange(
                    "p (t w) -> p t w", w=128)[:, :, 0:D]
                nc.vector.tensor_tensor(
                    tview, agk, rinv[:].to_broadcast([128, TCH, D]),
                    op=OP.mult)

                # ---- interleaved phase 2: accumulate agg.T += XF_r^T A_r --
                for r12 in range(TCH):
                    r = c * TCH + r12
                    ar = apool.tile([128, NPAD], BF16, tag="ar")
                    nc.sync.dma_start(ar[:],
                                      amat_in[:, r * NPAD:(r + 1) * NPAD])
                    for s in range(3):
                        nc.tensor.matmul(
                            accps[s][:], table[:, r * 128:r * 128 + D],
                            ar[:, s * 512:(s + 1) * 512],
                            start=(r == 0), stop=(r == NRANKS - 1))

            # ================= Phase 2 tail: transpose + normalize ========
            accb = cpool.tile([D, NPAD], BF16)
            for s in range(3):
                nc.vector.tensor_copy(accb[:, s * 512:(s + 1) * 512],
                                      accps[s][:])
            acc = cpool.tile([128, NT * D], F32)
            for t in range(NT):
                tp = tpool.tile([128, D], BF16, tag="p2_tp")
                nc.tensor.transpose(tp[:], accb[:, t * 128:(t + 1) * 128],
                                    ident[:])
                nc.scalar.copy(acc[:, t * D:(t + 1) * D], tp[:])

            sqa2 = spool.tile([128, NT * D], F32, tag="q_sqa")
            nc.scalar.square(sqa2[:], acc[:])
            sqa2k = sqa2[:].rearrange("p (t o) -> p t o", o=D)
            sp22 = spool.tile([128, NT], F32, tag="q_sp2")
            nc.vector.tensor_reduce(sp22[:], sqa2k[:, :, 1:], axis=AX.X,
                                    op=OP.add)
            inner2 = spool.tile([128, NT], F32, tag="q_inner")
            nc.vector.tensor_tensor(inner2[:], sp22[:], sqa2k[:, :, 0],
                                    op=OP.subtract)
            nc.vector.scalar_tensor_tensor(
                inner2[:], inner2[:], -1.0, inner2[:], op0=OP.mult, op1=OP.max)
            nc.vector.tensor_scalar_max(inner2[:], inner2[:], 1e-8)
            rden2 = spool.tile([128, NT], F32, tag="q_rden")
            nc.scalar.sqrt(rden2[:], inner2[:])
            rinv2 = spool.tile([128, NT], F32, tag="q_rinv")
            nc.vector.reciprocal(rinv2[:], rden2[:])
            outb = spool.tile([128, NT * D], F32, tag="q_out")
            nc.vector.tensor_tensor(
                outb[:].rearrange("p (t o) -> p t o", o=D),
                acc[:].rearrange("p (t o) -> p t o", o=D),
                rinv2[:].to_broadcast([128, NT, D]), op=OP.mult)

            nc.sync.dma_start(
                out_dram[:].rearrange("(t p) d -> p t d", p=128),
                outb[:].rearrange("p (t d) -> p t d", t=NT))

    nc.compile()
    return nc


_NC_CACHE = None


def _get_nc():
    global _NC_CACHE
    if _NC_CACHE is None:
        _NC_CACHE = _build_program()
    return _NC_CACHE


def _host_inputs(x, nei, nei_mask, kernel_points, W, b, scales):
    x = np.asarray(x, np.float32)
    nei = np.asarray(nei, np.int64)
    nei_mask = np.asarray(nei_mask)
    kernel_points = np.asarray(kernel_points, np.float32)
    W = np.asarray(W, np.float32)
    b = np.asarray(b, np.float32)
    scales = np.asarray(scales, np.float32)

    # kernels = expmap0(kernel_points), then fold the Lorentz metric
    sp = kernel_points[:, 1:]
    nrm = np.sqrt(np.clip(np.sum(sp * sp, -1, keepdims=True), 1e-8, None)
                  ).astype(np.float32)
    kernels = np.concatenate(
        [np.cosh(nrm), np.sinh(nrm) / nrm * sp], -1).astype(np.float32)
    metric = np.ones((D,), np.float32)
    metric[1:] = -1.0
    km = kernels * metric                      # [K, D]

    xp = np.concatenate([x, x[:NPF - N]], 0)   # [12288, 64]
    xaugT = np.concatenate(
        [xp.T, np.ones((1, NPF), np.float32)], 0).astype(np.float32)

    wtb = np.concatenate(
        [np.transpose(W, (2, 0, 1)).reshape(D, KD), b.reshape(1, KD)],
        0).astype(np.float32)                  # [65, 512]
    kmtb = np.concatenate([km.T, np.zeros((1, K), np.float32)], 0)  # [65, 8]
    es96 = np.tile(np.exp(scales).astype(np.float32), TCH)[None, :].repeat(
        128, 0).copy()                         # [128, 96]
    ident = np.eye(D, dtype=np.float32)

    # Mask folded into the 0/1-count matrix A[j, n_local]; all-masked nodes
    # keep every edge (equal weights == the reference's uniform-1e-4
    # midpoint; the final normalize cancels the scale).
    mask = nei_mask.astype(bool)
    allm = ~mask.any(1)
    eff = mask | allm[:, None]                 # [N, NEI]

    import ml_dtypes
    shared = {"xaugbf": np.ascontiguousarray(xaugT.astype(ml_dtypes.bfloat16)),
              "wtbbf": wtb.astype(ml_dtypes.bfloat16),
              "kmtb": kmtb.astype(ml_dtypes.bfloat16), "es96": es96,
              "ident": ident.astype(ml_dtypes.bfloat16)}
    per_core = []
    for r in range(NCORES):
        nj = nei[r * SLICE:(r + 1) * SLICE]          # [1500, 32]
        ef = eff[r * SLICE:(r + 1) * SLICE]
        nloc = np.repeat(np.arange(SLICE), NEI)
        jsrc = nj.reshape(-1)
        keep = ef.reshape(-1)
        A = np.zeros((NPF, NPAD), np.float32)
        np.add.at(A, (jsrc[keep], nloc[keep]), 1.0)
        amat = np.ascontiguousarray(
            A.reshape(NRANKS, 128, NPAD).transpose(1, 0, 2).reshape(
                128, NRANKS * NPAD).astype(ml_dtypes.bfloat16))
        per_core.append({**shared, "amat": amat})
    return per_core


def kernel(x, nei, nei_mask, kernel_points, W, b, scales, _trace=False):
    nc = _get_nc()
    per_core = _host_inputs(x, nei, nei_mask, kernel_points, W, b, scales)
    res = run_bass_kernel_spmd(nc, per_core, core_ids=list(range(NCORES)),
                               trace=_trace)
    out = np.concatenate(
        [res.results[r]["out"][:SLICE] for r in range(NCORES)], 0)
    if _trace:
        kernel.last_result = res
    return out.astype(np.float32)


# revision 21
# speedup vs baseline: 1.0253x; 1.0199x over previous
"""Trainium2 Bass kernel for KernelPointAggregation (hyperbolic GNN message passing).

v4 strategy: SBUF XF table + one-hot-matmul neighbor aggregation (no gather).
-----------------------------------------------------------------------------
Every per-edge quantity factorizes per source node j = nei[n, m]: the kernel
is a per-node table XF[j, :] followed by a masked sum over neighbors and a
Lorentz normalize (the normalize cancels any per-node weight scale, so the
{1e-4, 1.0001} mask weights reduce to {drop, keep} -> a 0/1-count matrix A).

The neighbor aggregation agg = A^T @ XF runs on the (otherwise idle) tensor
engine, accumulated rank-by-rank INSIDE phase 1: as soon as chunk c writes
table ranks 12c..12c+11, the PE contracts those ranks against the streamed
A rows, so phase 2 costs no extra wall time. A (12288 x 1536 bf16 per core,
~38 MB) streams from HBM overlapped with phase-1 compute.

Per core:
  Phase 1 (replicated): 8 chunks x 12 tiles x 128 nodes:
    y[j,(k,o)] = [x_j,1] @ wtb,  nis[j,k] = [x_j,1] @ kmtb,
    kernel-softmax collapses to u^-1 = z - sqrt(z^2-1) (denominator cancels),
    k-aggregation + Lorentz normalize -> XF row -> SBUF table bf16
    (partition = j%128, rank j//128; cols 64..127 of each rank block = 0).
  Interleaved per chunk: for each new rank r, 3 matmuls accumulate
    psum[64, 512-slice] += table[:, r-block]^T @ A[r-block rows].
  Tail: cast psum -> bf16, PE-transpose [64, n] -> [n, 64], Lorentz midpoint
    normalize, DMA out.
"""

import sys

sys.path.insert(0, "/opt/trn_rl_repo")

import math
import os

import numpy as np

import concourse.bacc as bacc
import concourse.mybir as mybir
import concourse.tile as tile
from concourse.bass_utils import run_bass_kernel_spmd

N, NEI, K, D = 12000, 32, 8, 64
NCORES = 8
SLICE = N // NCORES          # 1500 nodes per core (phase 2 ownership)
NT = 12                      # node tiles per core slice (128 each)
NPAD = NT * 128              # 1536 padded rows per core slice
NPF = 12288                  # padded full node count (96 ranks of 128)
NRANKS = 96
NCHUNK = 6                   # phase-1 chunks
TCH = 16                     # tiles per chunk
CH = TCH * 128               # 1536 nodes per chunk
KD = K * D                   # 512
S = TCH * K                  # 96 (t,k) pairs per chunk
HS = S // 2                  # 48
HKD = TCH * KD // 2          # 3072
SG = 106                     # DVE (t,k) groups of S=128; GpSimd takes 22
SPL = SG * D                 # 5120
F32 = mybir.dt.float32
BF16 = mybir.dt.bfloat16
AX = mybir.AxisListType
OP = mybir.AluOpType
AF = mybir.ActivationFunctionType


def _build_program():
    nc = bacc.Bacc("TRN2", target_bir_lowering=False, debug=False,
                   num_devices=NCORES)

    xaugbf_in = nc.dram_tensor("xaugbf", [D + 1, NPF], BF16,
                               kind="ExternalInput")
    wtb_in = nc.dram_tensor("wtbbf", [D + 1, KD], BF16, kind="ExternalInput")
    kmtb_in = nc.dram_tensor("kmtb", [D + 1, K], BF16, kind="ExternalInput")
    es_in = nc.dram_tensor("es96", [128, S], F32, kind="ExternalInput")
    amat_in = nc.dram_tensor("amat", [128, NRANKS * NPAD], BF16,
                             kind="ExternalInput")
    id_in = nc.dram_tensor("ident", [D, D], BF16, kind="ExternalInput")
    out_dram = nc.dram_tensor("out", [NPAD, D], F32, kind="ExternalOutput")

    with tile.TileContext(nc) as tc:
        with (
            tc.tile_pool(name="const", bufs=1) as cpool,
            tc.tile_pool(name="xa", bufs=3) as xpool,
            tc.tile_pool(name="amat", bufs=4) as apool,
            tc.tile_pool(name="yb", bufs=2) as bpool,
            tc.tile_pool(name="sq", bufs=2) as sqpool,
            tc.tile_pool(name="tm", bufs=1) as tmpool,
            tc.tile_pool(name="scr", bufs=2) as spool,
            tc.tile_pool(name="nrm", bufs=2) as nrmpool,
            tc.tile_pool(name="psum", bufs=1, space="PSUM") as ppool,
            tc.tile_pool(name="psumn", bufs=1, space="PSUM") as npool,
            tc.tile_pool(name="psuma", bufs=1, space="PSUM") as acpool,
            tc.tile_pool(name="psumt", bufs=2, space="PSUM") as tpool,
        ):
            # ---- constants ----
            wtb = cpool.tile([D + 1, KD], BF16)
            nc.sync.dma_start(wtb[:], wtb_in[:])
            kmtb = cpool.tile([D + 1, K], BF16)
            nc.sync.dma_start(kmtb[:], kmtb_in[:])
            es96 = cpool.tile([128, S], F32)
            nc.sync.dma_start(es96[:], es_in[:])
            ident = cpool.tile([D, D], BF16)
            nc.sync.dma_start(ident[:], id_in[:])

            # SBUF XF table: partition = j%128, rank j//128 at 256B stride;
            # cols 0..63 of each 128-wide rank block = XF row, 64..127 = 0.
            table = cpool.tile([128, NRANKS * 128], BF16)
            nc.vector.memset(table[:], 0.0)

            # phase-2 accumulators: agg.T[64, 1536] in 3 psum banks
            accps = [acpool.tile([D, 512], F32, tag=f"acc{s}",
                                 name=f"accps{s}")
                     for s in range(3)]

            # ================= Phase 1 + interleaved A-matmul ============
            for c in range(NCHUNK):
                xaugbf = xpool.tile([D + 1, CH], BF16, tag="xaugbf")
                nc.sync.dma_start(xaugbf[:],
                                  xaugbf_in[:, c * CH:(c + 1) * CH])
                ybuf = bpool.tile([128, TCH * KD], BF16, tag="ybuf")
                sqb = sqpool.tile([128, TCH * KD], BF16, tag="sqb")
                nis = spool.tile([128, S], F32, tag="nis")
                ssqn = spool.tile([128, S], F32, tag="ssqn")
                sig = spool.tile([128, S], F32, tag="sig")
                nipb = npool.tile([128, S], F32, tag="nip")

                for tt in range(0, TCH, 2):
                    yp2 = ppool.tile([128, 2 * KD], F32, tag="yp")
                    for u in range(2):
                        t = tt + u
                        nc.tensor.matmul(yp2[:, u * KD:(u + 1) * KD],
                                         xaugbf[:, t * 128:(t + 1) * 128],
                                         wtb[:], start=True, stop=True)
                        nc.tensor.matmul(nipb[:, t * K:(t + 1) * K],
                                         xaugbf[:, t * 128:(t + 1) * 128],
                                         kmtb[:], start=True, stop=True)
                    nc.scalar.copy(ybuf[:, tt * KD:(tt + 2) * KD], yp2[:])
                nc.vector.tensor_copy(nis[:], nipb[:])

                # squares + segmented ||nar||^2, split DVE / GpSimd halves
                nc.vector.tensor_tensor(sqb[:, :SPL], ybuf[:, :SPL],
                                        ybuf[:, :SPL], op=OP.mult)
                nc.gpsimd.tensor_tensor(sqb[:, SPL:], ybuf[:, SPL:],
                                        ybuf[:, SPL:], op=OP.mult)
                sqbk = sqb[:].rearrange("p (s o) -> p s o", s=S)
                ssqa = spool.tile([128, S], F32, tag="ssqa")
                nc.vector.tensor_reduce(ssqa[:], sqbk, axis=AX.X, op=OP.add)
                nc.vector.tensor_tensor(ssqn[:], ssqa[:], sqbk[:, :, 0],
                                        op=OP.subtract)

                ybk = ybuf[:].rearrange("p (s o) -> p s o", s=S)
                nc.scalar.activation(sig[:], ybk[:, :, 0], AF.Sigmoid)
                # ---- batched nonlinear over [128, 96] ----
                # 1/u = 1/(z+sqrt(z^2-1)) = z - sqrt(z^2-1)
                z = spool.tile([128, S], F32, tag="pb1")
                nc.vector.tensor_scalar_max(z[:], nis[:], 1.0 + 1e-7)
                zsq = spool.tile([128, S], F32, tag="pb2")
                nc.vector.tensor_tensor(zsq[:], z[:], z[:], op=OP.mult)
                nc.vector.tensor_scalar_sub(zsq[:], zsq[:], 1.0)
                sq1 = spool.tile([128, S], F32, tag="pb3")
                nc.scalar.activation(sq1[:], zsq[:], AF.Sqrt)
                uinv = spool.tile([128, S], F32, tag="pb5")
                nc.vector.tensor_tensor(uinv[:], z[:], sq1[:],
                                        op=OP.subtract)
                tt2 = spool.tile([128, S], F32, tag="pb9")
                nc.vector.tensor_tensor(tt2[:], sig[:], es96[:], op=OP.mult)
                nc.vector.tensor_scalar_add(tt2[:], tt2[:], 1.0001)
                num = spool.tile([128, S], F32, tag="pb10")
                nc.vector.tensor_tensor(num[:], tt2[:], tt2[:], op=OP.mult)
                nc.vector.tensor_scalar_sub(num[:], num[:], 1.0)
                dnn = spool.tile([128, S], F32, tag="pb11")
                nc.vector.tensor_scalar_max(dnn[:], ssqn[:], 1e-8)
                dni = spool.tile([128, S], F32, tag="pb12")
                nc.vector.reciprocal(dni[:], dnn[:])
                sc = spool.tile([128, S], F32, tag="pb13")
                nc.vector.tensor_tensor(sc[:], num[:], dni[:], op=OP.mult)
                rt = spool.tile([128, S], F32, tag="pb14")
                nc.scalar.activation(rt[:], sc[:], AF.Sqrt)
                csp = spool.tile([128, S], BF16, tag="pb15")
                nc.vector.tensor_tensor(csp[:], uinv[:], rt[:], op=OP.mult)
                wt = spool.tile([128, S], F32, tag="pb16")
                nc.vector.tensor_tensor(wt[:], uinv[:], tt2[:], op=OP.mult)
                aggt = spool.tile([128, TCH], F32, tag="pb17")
                nc.vector.tensor_reduce(
                    aggt[:], wt[:].rearrange("p (t k) -> p t k", k=K),
                    axis=AX.X, op=OP.add)

                # ---- k-aggregation: weight, then pairwise-add over k ----
                tmpb = tmpool.tile([128, TCH * KD], BF16, tag="tmpb")
                nc.vector.tensor_tensor(
                    tmpb[:, :SPL].rearrange("p (s o) -> p s o", s=SG),
                    ybk[:, :SG, :],
                    csp[:, :SG].to_broadcast([128, SG, D]), op=OP.mult)
                nc.gpsimd.tensor_tensor(
                    tmpb[:, SPL:].rearrange("p (s o) -> p s o", s=S - SG),
                    ybk[:, SG:, :],
                    csp[:, SG:].to_broadcast([128, S - SG, D]), op=OP.mult)
                tk = tmpb[:].rearrange("p (t k o) -> p t k o", t=TCH, k=K)
                a1 = sqpool.tile([128, TCH * 4 * D], BF16, tag="a1")
                a1k = a1[:].rearrange("p (t k o) -> p t k o", t=TCH, k=4)
                nc.vector.tensor_tensor(a1k, tk[:, :, 0:4, :],
                                        tk[:, :, 4:8, :], op=OP.add)
                a2 = sqpool.tile([128, TCH * 2 * D], BF16, tag="a2")
                a2k = a2[:].rearrange("p (t k o) -> p t k o", t=TCH, k=2)
                nc.vector.tensor_tensor(a2k, a1k[:, :, 0:2, :],
                                        a1k[:, :, 2:4, :], op=OP.add)
                agg = nrmpool.tile([128, TCH * D], F32, tag="agg")
                agk = agg[:].rearrange("p (t o) -> p t o", o=D)
                nc.vector.tensor_tensor(agk, a2k[:, :, 0, :],
                                        a2k[:, :, 1, :], op=OP.add)
                nc.vector.tensor_copy(agk[:, :, 0], aggt[:])

                # ---- Lorentz normalize, write bf16 into SBUF table ----
                sqa = nrmpool.tile([128, TCH * D], F32, tag="sqa")
                nc.scalar.square(sqa[:], agg[:])
                sqak = sqa[:].rearrange("p (t o) -> p t o", o=D)
                sp2 = spool.tile([128, TCH], F32, tag="sp2")
                nc.vector.tensor_reduce(sp2[:], sqak, axis=AX.X, op=OP.add)
                inner = spool.tile([128, TCH], F32, tag="inner")
                nc.vector.scalar_tensor_tensor(
                    inner[:], sqak[:, :, 0], -2.0, sp2[:],
                    op0=OP.mult, op1=OP.add)
                nc.vector.scalar_tensor_tensor(
                    inner[:], inner[:], -1.0, inner[:],
                    op0=OP.mult, op1=OP.max)
                nc.vector.tensor_scalar_max(inner[:], inner[:], 1e-8)
                rden = spool.tile([128, TCH], F32, tag="rden")
                nc.scalar.sqrt(rden[:], inner[:])
                rinv = spool.tile([128, TCH], F32, tag="rinv")
                nc.vector.reciprocal(rinv[:], rden[:])
                tview = table[:, c * TCH * 128:(c + 1) * TCH * 128].rearrange(
                    "p (t w) -> p t w", w=128)[:, :, 0:D]
                nc.vector.tensor_tensor(
                    tview, agk, rinv[:].to_broadcast([128, TCH, D]),
                    op=OP.mult)

                # ---- interleaved phase 2: accumulate agg.T += XF_r^T A_r --
                for r12 in range(TCH):
                    r = c * TCH + r12
                    ar = apool.tile([128, NPAD], BF16, tag="ar")
                    nc.sync.dma_start(ar[:],
                                      amat_in[:, r * NPAD:(r + 1) * NPAD])
                    for s in range(3):
                        nc.tensor.matmul(
                            accps[s][:], table[:, r * 128:r * 128 + D],
                            ar[:, s * 512:(s + 1) * 512],
                            start=(r == 0), stop=(r == NRANKS - 1))

            # ================= Phase 2 tail: transpose + normalize ========
            accb = cpool.tile([D, NPAD], BF16)
            for s in range(3):
                nc.vector.tensor_copy(accb[:, s * 512:(s + 1) * 512],
                                      accps[s][:])
            acc = cpool.tile([128, NT * D], F32)
            for t in range(NT):
                tp = tpool.tile([128, D], BF16, tag="p2_tp")
                nc.tensor.transpose(tp[:], accb[:, t * 128:(t + 1) * 128],
                                    ident[:])
                nc.scalar.copy(acc[:, t * D:(t + 1) * D], tp[:])

            sqa2 = spool.tile([128, NT * D], F32, tag="q_sqa")
            nc.scalar.square(sqa2[:], acc[:])
            sqa2k = sqa2[:].rearrange("p (t o) -> p t o", o=D)
            sp22 = spool.tile([128, NT], F32, tag="q_sp2")
            nc.vector.tensor_reduce(sp22[:], sqa2k[:, :, 1:], axis=AX.X,
                                    op=OP.add)
            inner2 = spool.tile([128, NT], F32, tag="q_inner")
            nc.vector.tensor_tensor(inner2[:], sp22[:], sqa2k[:, :, 0],
                                    op=OP.subtract)
            nc.vector.scalar_tensor_tensor(
                inner2[:], inner2[:], -1.0, inner2[:], op0=OP.mult, op1=OP.max)
            nc.vector.tensor_scalar_max(inner2[:], inner2[:], 1e-8)
            rden2 = spool.tile([128, NT], F32, tag="q_rden")
            nc.scalar.sqrt(rden2[:], inner2[:])
            rinv2 = spool.tile([128, NT], F32, tag="q_rinv")
            nc.vector.reciprocal(rinv2[:], rden2[:])
            outb = spool.tile([128, NT * D], F32, tag="q_out")
            nc.vector.tensor_tensor(
                outb[:].rearrange("p (t o) -> p t o", o=D),
                acc[:].rearrange("p (t o) -> p t o", o=D),
                rinv2[:].to_broadcast([128, NT, D]), op=OP.mult)

            nc.sync.dma_start(
                out_dram[:].rearrange("(t p) d -> p t d", p=128),
                outb[:].rearrange("p (t d) -> p t d", t=NT))

    nc.compile()
    return nc


_NC_CACHE = None


def _get_nc():
    global _NC_CACHE
    if _NC_CACHE is None:
        _NC_CACHE = _build_program()
    return _NC_CACHE


def _host_inputs(x, nei, nei_mask, kernel_points, W, b, scales):
    x = np.asarray(x, np.float32)
    nei = np.asarray(nei, np.int64)
    nei_mask = np.asarray(nei_mask)
    kernel_points = np.asarray(kernel_points, np.float32)
    W = np.asarray(W, np.float32)
    b = np.asarray(b, np.float32)
    scales = np.asarray(scales, np.float32)

    # kernels = expmap0(kernel_points), then fold the Lorentz metric
    sp = kernel_points[:, 1:]
    nrm = np.sqrt(np.clip(np.sum(sp * sp, -1, keepdims=True), 1e-8, None)
                  ).astype(np.float32)
    kernels = np.concatenate(
        [np.cosh(nrm), np.sinh(nrm) / nrm * sp], -1).astype(np.float32)
    metric = np.ones((D,), np.float32)
    metric[1:] = -1.0
    km = kernels * metric                      # [K, D]

    xp = np.concatenate([x, x[:NPF - N]], 0)   # [12288, 64]
    xaugT = np.concatenate(
        [xp.T, np.ones((1, NPF), np.float32)], 0).astype(np.float32)

    wtb = np.concatenate(
        [np.transpose(W, (2, 0, 1)).reshape(D, KD), b.reshape(1, KD)],
        0).astype(np.float32)                  # [65, 512]
    kmtb = np.concatenate([km.T, np.zeros((1, K), np.float32)], 0)  # [65, 8]
    es96 = np.tile(np.exp(scales).astype(np.float32), TCH)[None, :].repeat(
        128, 0).copy()                         # [128, 96]
    ident = np.eye(D, dtype=np.float32)

    # Mask folded into the 0/1-count matrix A[j, n_local]; all-masked nodes
    # keep every edge (equal weights == the reference's uniform-1e-4
    # midpoint; the final normalize cancels the scale).
    mask = nei_mask.astype(bool)
    allm = ~mask.any(1)
    eff = mask | allm[:, None]                 # [N, NEI]

    import ml_dtypes
    shared = {"xaugbf": np.ascontiguousarray(xaugT.astype(ml_dtypes.bfloat16)),
              "wtbbf": wtb.astype(ml_dtypes.bfloat16),
              "kmtb": kmtb.astype(ml_dtypes.bfloat16), "es96": es96,
              "ident": ident.astype(ml_dtypes.bfloat16)}
    per_core = []
    for r in range(NCORES):
        nj = nei[r * SLICE:(r + 1) * SLICE]          # [1500, 32]
        ef = eff[r * SLICE:(r + 1) * SLICE]
        nloc = np.repeat(np.arange(SLICE), NEI)
        jsrc = nj.reshape(-1)
        keep = ef.reshape(-1)
        A = np.zeros((NPF, NPAD), np.float32)
        np.add.at(A, (jsrc[keep], nloc[keep]), 1.0)
        amat = np.ascontiguousarray(
            A.reshape(NRANKS, 128, NPAD).transpose(1, 0, 2).reshape(
                128, NRANKS * NPAD).astype(ml_dtypes.bfloat16))
        per_core.append({**shared, "amat": amat})
    return per_core


def kernel(x, nei, nei_mask, kernel_points, W, b, scales, _trace=False):
    nc = _get_nc()
    per_core = _host_inputs(x, nei, nei_mask, kernel_points, W, b, scales)
    res = run_bass_kernel_spmd(nc, per_core, core_ids=list(range(NCORES)),
                               trace=_trace)
    out = np.concatenate(
        [res.results[r]["out"][:SLICE] for r in range(NCORES)], 0)
    if _trace:
        kernel.last_result = res
    return out.astype(np.float32)
